# revision 14
# baseline (speedup 1.0000x reference)
"""Trainium2 Bass kernel for nn_DecoderLSTMCell.

Computes, for B=16384 rows:
    gates = y @ W.T + h0 @ U.T + ctx @ C.T + b            # [B, 4H]
    i, f, o, g = split(gates, 4); i,f,o = sigmoid; g = tanh
    c = i * g + f * c0 ; h = o * tanh(c)
Returns (c, h), both [B, H] float32.

Strategy: data-parallel over the batch dim across 8 NeuronCores (2048
rows/core), weights replicated.  Per core the gate GEMM is
[M=2048, K=4096] @ [K=4096, N=4096] — computed via ONE level of
Strassen with fp16 operands:

  * fp16 runs the tensor engine at the same 1 cycle/row as bf16 but with
    8x less rounding noise, which buys the error headroom Strassen needs
    (measured ~2.4e-3 max rel err vs the 2e-2 gate; plain bf16 is 8e-3).
  * Strassen does 7 half-size products instead of 8: 3584 matmul
    instructions instead of 4096, i.e. 7/8 of the tensor-engine time,
    which is the bottleneck (96% busy in the direct kernel).

The 7 S (x-side) and 7 T (weight-side) block combinations are formed on
the host in fp32 and shipped as fp16 — free accuracy and zero device
cost.  On-device, each (nb, mt-pair) unit accumulates the 7 products
into four SBUF gate tiles (C11/C12/C21/C22) via DVE adds with the bias
folded into the first touch, then runs the LSTM epilogue per gate tile
as soon as its last product lands.
"""

import numpy as np

import concourse.tile as tile
import concourse.mybir as mybir
from concourse import bacc, bass_utils

P = 128
F32 = mybir.dt.float32
F16 = mybir.dt.float16
AF = mybir.ActivationFunctionType

# Problem shapes (hardcoded; see module docstring)
B, IN, H, CTX = 16384, 1024, 1024, 2048
KD = IN + H + CTX  # 4096 contraction dim
NCORES = 8
BC = B // NCORES  # 2048 batch rows per core
MH = BC // 2      # 1024 = Strassen row-block
KH = KD // 2      # 2048 = Strassen contraction block
KT2 = KH // P     # 16 k-tiles per product
NB = 4            # 512-wide gate blocks per N-half
PT = 4            # mt-pairs per row-half
J = 7             # Strassen products

# accumulation plan: per product j, list of (Cname, sign, is_first_touch);
# epilogue fires at each C tile's last touch.
#   C11 = P0+P3-P4+P6 ; C12 = P2+P4 ; C21 = P1+P3 ; C22 = P0-P1+P2+P5
ACC = [
    [("c11", 1, True), ("c22", 1, True)],
    [("c21", 1, True), ("c22", -1, False)],
    [("c12", 1, True), ("c22", 1, False)],
    [("c11", 1, False), ("c21", 1, False)],
    [("c12", 1, False), ("c11", -1, False)],
    [("c22", 1, False)],
    [("c11", 1, False)],
]
LAST_TOUCH = {"c21": 3, "c12": 4, "c22": 5, "c11": 6}
# (row_half, e_half) per C name: rows = row_half*MH + mt*P, e = e_half*4 + nb
CPOS = {"c11": (0, 0), "c21": (1, 0), "c12": (0, 1), "c22": (1, 1)}

LAST_RESULT = None  # BassKernelResults of the most recent run (for test.py)
_NC_CACHE = None  # compiled Bass module, reused across kernel() calls


def _ksplits(kt, first, fine=False):
    """k-tile DMA split sizes; fine-grained ramp on the critical first load.
    `fine` keeps chunks small so warmup stalls stay under the ~3.4us HAM MID
    window (a longer PE idle re-throttles the clock gate to 1.2 GHz)."""
    if first:
        return [1, 1, 2, 4] + [2] * ((kt - 8) // 2)
    if fine:
        return [2] * (kt // 2)
    return [kt // 4] * 4


def build_nc():
    nc = bacc.Bacc("TRN2", target_bir_lowering=False)
    sTh = nc.dram_tensor("sTh", (J, PT, P, KT2, 2 * P), F16, kind="ExternalInput")
    tTh = nc.dram_tensor("tTh", (J, NB, P, KT2, 4 * P), F16, kind="ExternalInput")
    # c0 / outputs in blocked [mt, e, 128, 128] layout: contiguous 64KB DMA
    # tiles (vs 512B strided bursts of the flat [BC, H] layout)
    c0b = nc.dram_tensor("c0b", (2 * PT * 2, 8, P, P), F32, kind="ExternalInput")
    bb = nc.dram_tensor("bb", (P, 4 * H), F32, kind="ExternalInput")
    c_out = nc.dram_tensor("c_out", (2 * PT * 2, 8, P, P), F32, kind="ExternalOutput")
    h_out = nc.dram_tensor("h_out", (2 * PT * 2, 8, P, P), F32, kind="ExternalOutput")
    NW = 4 * P  # 512: one [i|f|o|g] gate block

    with (
        tile.TileContext(nc) as tc,
        tc.tile_pool(name="tp", bufs=1) as tp,
        tc.tile_pool(name="sp", bufs=1) as sp,
        tc.tile_pool(name="cp", bufs=1) as cp,
        tc.tile_pool(name="bp", bufs=1) as bp,
        tc.tile_pool(name="c0p", bufs=1) as c0p,
        tc.tile_pool(name="gp", bufs=3) as gp,
        tc.tile_pool(name="ep", bufs=3) as ep,
        tc.tile_pool(name="pp", bufs=8, space="PSUM") as pp,
    ):
        def epilogue(cname, ct_g, c0_t, bmt, e):
            act = gp.tile([P, NW], F32, tag="act", name=f"act_{cname}_{bmt}_{e}")
            nc.scalar.activation(act[:, 0:3 * P], ct_g[:, 0:3 * P], AF.Sigmoid)
            nc.scalar.activation(act[:, 3 * P:4 * P], ct_g[:, 3 * P:4 * P], AF.Tanh)
            ct = ep.tile([P, P], F32, tag="ct", name=f"ct_{bmt}_{e}")
            nc.vector.tensor_mul(ct[:], act[:, 0:P], act[:, 3 * P:4 * P])
            fc = ep.tile([P, P], F32, tag="fc", name=f"fc_{bmt}_{e}")
            nc.vector.tensor_mul(fc[:], act[:, P:2 * P], c0_t[:])
            nc.vector.tensor_add(ct[:], ct[:], fc[:])
            nc.scalar.dma_start(out=c_out[bmt, e], in_=ct[:])
            tct = ep.tile([P, P], F32, tag="tct", name=f"tct_{bmt}_{e}")
            nc.scalar.activation(tct[:], ct[:], AF.Tanh)
            ht = ep.tile([P, P], F32, tag="ht", name=f"ht_{bmt}_{e}")
            nc.vector.tensor_mul(ht[:], act[:, 2 * P:3 * P], tct[:])
            nc.scalar.dma_start(out=h_out[bmt, e], in_=ht[:])

        s_cache = {}  # j -> (pt, tile): S tiles still resident from prior nb
        for nb in range(NB):
            first_nb = nb == 0
            tt = []
            for j in range(J):
                t = tp.tile([P, KT2, NW], F16, tag=f"tt{j}", name=f"tt_{nb}_{j}")
                q = 0
                for sz in _ksplits(KT2, first=(first_nb and j == 0), fine=first_nb):
                    nc.sync.dma_start(out=t[:, q:q + sz], in_=tTh[j, nb, :, q:q + sz])
                    q += sz
                tt.append(t)
            # zigzag pt order: the boundary pt's S tiles stay resident across nb
            pts = list(range(PT)) if nb % 2 == 0 else list(range(PT - 1, -1, -1))
            for pt in pts:
                first_unit = first_nb and pt == 0
                st = {}
                biases = {}
                c0t = {}

                def load_s(j):
                    if s_cache.get(j, (None, None))[0] == pt:
                        st[j] = s_cache[j][1]
                        return
                    s = sp.tile([P, KT2, 2 * P], F16, tag=f"st{j}", name=f"st_{nb}_{pt}_{j}")
                    q = 0
                    for sz in _ksplits(KT2, first=(first_unit and j == 0), fine=first_unit):
                        nc.scalar.dma_start(out=s[:, q:q + sz], in_=sTh[j, pt, :, q:q + sz])
                        q += sz
                    st[j] = s
                    s_cache[j] = (pt, s)

                def load_bias():
                    for half, tag in ((0, "blo"), (1, "bhi")):
                        e = half * 4 + nb
                        bt = bp.tile([P, NW], F32, tag=tag, name=f"bias_{nb}_{half}")
                        nc.scalar.dma_start(out=bt[:], in_=bb[:, e * NW:(e + 1) * NW])
                        biases[half] = bt

                def load_c0():
                    for mt2 in (0, 1):
                        mt = pt * 2 + mt2
                        for cname, (rh, eh) in CPOS.items():
                            bmt = rh * (2 * PT) + mt
                            e = eh * 4 + nb
                            t = c0p.tile([P, P], F32, tag=f"c0_{cname}_{mt2}",
                                         name=f"c0_{nb}_{pt}_{cname}_{mt2}")
                            nc.sync.dma_start(out=t[:], in_=c0b[bmt, e])
                            c0t[(cname, mt2)] = t

                # DMA issue order: critical S first, then bias (needed at the
                # first accum), then c0 (needed at the first epilogue).
                load_s(0)
                if pt == pts[0]:
                    load_bias()
                else:
                    biases = last_biases  # noqa: F821
                load_s(1)
                load_s(2)
                load_c0()
                for j in range(3, J):
                    load_s(j)
                last_biases = biases

                C = {}
                for j in range(J):
                    for mt2 in (0, 1):
                        mt = pt * 2 + mt2
                        ps = pp.tile([P, NW], F32, tag="ps", name=f"ps_{nb}_{pt}_{j}_{mt2}")
                        for k in range(KT2):
                            nc.tensor.matmul(
                                ps[:],
                                st[j][:, k, mt2 * P:(mt2 + 1) * P],
                                tt[j][:, k, :],
                                start=(k == 0),
                                stop=(k == KT2 - 1),
                            )
                        for cname, sign, first in ACC[j]:
                            key = (cname, mt2)
                            rh, eh = CPOS[cname]
                            if first:
                                ctile = cp.tile([P, NW], F32, tag=f"C{cname}_{mt2}",
                                                name=f"C_{nb}_{pt}_{cname}_{mt2}")
                                nc.vector.tensor_add(ctile[:], ps[:], biases[eh][:])
                                C[key] = ctile
                            elif sign > 0:
                                nc.vector.tensor_add(C[key][:], C[key][:], ps[:])
                            else:
                                nc.vector.tensor_sub(C[key][:], C[key][:], ps[:])
                        for cname, sign, first in ACC[j]:
                            if LAST_TOUCH[cname] == j:
                                key = (cname, mt2)
                                rh, eh = CPOS[cname]
                                bmt = rh * (2 * PT) + mt
                                e = eh * 4 + nb
                                epilogue(cname, C[key], c0t[key], bmt, e)
    nc.compile()
    return nc


def pack_inputs(y, ctx, c0, h0, W, U, C, b):
    """Host-side packing: Strassen S/T combos in fp32, rounded once to fp16."""
    x = np.concatenate([y, h0, ctx], axis=1)  # [B, KD]
    Wcat = np.concatenate([W, U, C], axis=1)  # [4H, KD]
    # packed gate column order: n = e*512 + gate*128 + u
    Bp = np.ascontiguousarray(
        Wcat.reshape(4, 8, P, KD).transpose(3, 1, 0, 2).reshape(KD, 4 * H)
    )
    B11, B12 = Bp[:KH, :2048], Bp[:KH, 2048:]
    B21, B22 = Bp[KH:, :2048], Bp[KH:, 2048:]
    T_list = [B11 + B22, B11, B12 - B22, B21 - B11, B22, B11 + B12, B21 + B22]
    tTh = np.empty((J, NB, P, KT2, 4 * P), dtype=np.float16)
    for j, T in enumerate(T_list):
        # T: [KH, 2048] -> [nb, p, kt, w]
        tTh[j] = T.reshape(KT2, P, NB, 4 * P).transpose(2, 1, 0, 3).astype(np.float16)

    sThs = []
    for ci in range(NCORES):
        xc = x[ci * BC:(ci + 1) * BC]
        A11, A12 = xc[:MH, :KH], xc[:MH, KH:]
        A21, A22 = xc[MH:, :KH], xc[MH:, KH:]
        S_list = [A11 + A22, A21 + A22, A11, A22, A11 + A12, A21 - A11, A12 - A22]
        sTh = np.empty((J, PT, P, KT2, 2 * P), dtype=np.float16)
        for j, S in enumerate(S_list):
            # S: [MH, KH] -> [pt, p, kt, u]
            sTh[j] = S.reshape(PT, 2 * P, KT2, P).transpose(0, 3, 2, 1).astype(np.float16)
        sThs.append(sTh)

    br = b.reshape(4, 8, P).transpose(1, 0, 2).reshape(4 * H)
    bb = np.ascontiguousarray(np.broadcast_to(br, (P, 4 * H)))

    c0bs = []
    for ci in range(NCORES):
        c0c = c0[ci * BC:(ci + 1) * BC]
        c0bs.append(np.ascontiguousarray(
            c0c.reshape(2 * PT * 2, P, 8, P).transpose(0, 2, 1, 3)
        ))
    return sThs, tTh, bb, c0bs


def kernel(y, ctx, c0, h0, W, U, C, b):
    global LAST_RESULT, _NC_CACHE
    y = np.ascontiguousarray(np.asarray(y, dtype=np.float32))
    ctx = np.ascontiguousarray(np.asarray(ctx, dtype=np.float32))
    c0 = np.ascontiguousarray(np.asarray(c0, dtype=np.float32))
    h0 = np.ascontiguousarray(np.asarray(h0, dtype=np.float32))
    W = np.ascontiguousarray(np.asarray(W, dtype=np.float32))
    U = np.ascontiguousarray(np.asarray(U, dtype=np.float32))
    C = np.ascontiguousarray(np.asarray(C, dtype=np.float32))
    b = np.ascontiguousarray(np.asarray(b, dtype=np.float32))

    sThs, tTh, bb, c0bs = pack_inputs(y, ctx, c0, h0, W, U, C, b)

    if _NC_CACHE is None:
        _NC_CACHE = build_nc()
    nc = _NC_CACHE
    in_maps = []
    for ci in range(NCORES):
        in_maps.append(
            {
                "sTh": sThs[ci],
                "tTh": tTh,
                "c0b": c0bs[ci],
                "bb": bb,
            }
        )
    res = bass_utils.run_bass_kernel_spmd(nc, in_maps, core_ids=list(range(NCORES)))
    LAST_RESULT = res

    def unblock(a):  # [16, 8, 128, 128] -> [2048, 1024]
        return a.transpose(0, 2, 1, 3).reshape(BC, H)

    c_full = np.concatenate([unblock(r["c_out"]) for r in res.results], axis=0)
    h_full = np.concatenate([unblock(r["h_out"]) for r in res.results], axis=0)
    return (c_full, h_full)


# revision 15
# speedup vs baseline: 1.0030x; 1.0030x over previous
"""Trainium2 Bass kernel for nn_DecoderLSTMCell.

Computes, for B=16384 rows:
    gates = y @ W.T + h0 @ U.T + ctx @ C.T + b            # [B, 4H]
    i, f, o, g = split(gates, 4); i,f,o = sigmoid; g = tanh
    c = i * g + f * c0 ; h = o * tanh(c)
Returns (c, h), both [B, H] float32.

Strategy: data-parallel over the batch dim across 8 NeuronCores (2048
rows/core), weights replicated.  Per core the gate GEMM is
[M=2048, K=4096] @ [K=4096, N=4096] — computed via ONE level of
Strassen with fp16 operands:

  * fp16 runs the tensor engine at the same 1 cycle/row as bf16 but with
    8x less rounding noise, which buys the error headroom Strassen needs
    (measured ~2.4e-3 max rel err vs the 2e-2 gate; plain bf16 is 8e-3).
  * Strassen does 7 half-size products instead of 8: 3584 matmul
    instructions instead of 4096, i.e. 7/8 of the tensor-engine time,
    which is the bottleneck (96% busy in the direct kernel).

The 7 S (x-side) and 7 T (weight-side) block combinations are formed on
the host in fp32 and shipped as fp16 — free accuracy and zero device
cost.  On-device, each (nb, mt-pair) unit accumulates the 7 products
into four SBUF gate tiles (C11/C12/C21/C22) via DVE adds with the bias
folded into the first touch, then runs the LSTM epilogue per gate tile
as soon as its last product lands.
"""

import numpy as np

import concourse.tile as tile
import concourse.mybir as mybir
from concourse import bacc, bass_utils

P = 128
F32 = mybir.dt.float32
F16 = mybir.dt.float16
AF = mybir.ActivationFunctionType

# Problem shapes (hardcoded; see module docstring)
B, IN, H, CTX = 16384, 1024, 1024, 2048
KD = IN + H + CTX  # 4096 contraction dim
NCORES = 8
BC = B // NCORES  # 2048 batch rows per core
MH = BC // 2      # 1024 = Strassen row-block
KH = KD // 2      # 2048 = Strassen contraction block
KT2 = KH // P     # 16 k-tiles per product
NB = 4            # 512-wide gate blocks per N-half
PT = 4            # mt-pairs per row-half
J = 7             # Strassen products

# accumulation plan: per product j, list of (Cname, sign, is_first_touch);
# epilogue fires at each C tile's last touch.
#   C11 = P0+P3-P4+P6 ; C12 = P2+P4 ; C21 = P1+P3 ; C22 = P0-P1+P2+P5
ACC = [
    [("c11", 1, True), ("c22", 1, True)],
    [("c21", 1, True), ("c22", -1, False)],
    [("c12", 1, True), ("c22", 1, False)],
    [("c11", 1, False), ("c21", 1, False)],
    [("c12", 1, False), ("c11", -1, False)],
    [("c22", 1, False)],
    [("c11", 1, False)],
]
LAST_TOUCH = {"c21": 3, "c12": 4, "c22": 5, "c11": 6}
# (row_half, e_half) per C name: rows = row_half*MH + mt*P, e = e_half*4 + nb
CPOS = {"c11": (0, 0), "c21": (1, 0), "c12": (0, 1), "c22": (1, 1)}

LAST_RESULT = None  # BassKernelResults of the most recent run (for test.py)
_NC_CACHE = None  # compiled Bass module, reused across kernel() calls


def _ksplits(kt, first, fine=False):
    """k-tile DMA split sizes; fine-grained ramp on the critical first load.
    `fine` keeps chunks small so warmup stalls stay under the ~3.4us HAM MID
    window (a longer PE idle re-throttles the clock gate to 1.2 GHz)."""
    if first:
        return [1, 1, 2, 4, kt - 8]
    if fine:
        return [4] * (kt // 4)
    return [kt // 4] * 4


def build_nc():
    nc = bacc.Bacc("TRN2", target_bir_lowering=False)
    sTh = nc.dram_tensor("sTh", (J, PT, P, KT2, 2 * P), F16, kind="ExternalInput")
    tTh = nc.dram_tensor("tTh", (J, NB, P, KT2, 4 * P), F16, kind="ExternalInput")
    # c0 / outputs in blocked [mt, e, 128, 128] layout: contiguous 64KB DMA
    # tiles (vs 512B strided bursts of the flat [BC, H] layout)
    c0b = nc.dram_tensor("c0b", (2 * PT * 2, 8, P, P), F32, kind="ExternalInput")
    bb = nc.dram_tensor("bb", (P, 4 * H), F32, kind="ExternalInput")
    c_out = nc.dram_tensor("c_out", (2 * PT * 2, 8, P, P), F32, kind="ExternalOutput")
    h_out = nc.dram_tensor("h_out", (2 * PT * 2, 8, P, P), F32, kind="ExternalOutput")
    NW = 4 * P  # 512: one [i|f|o|g] gate block

    with (
        tile.TileContext(nc) as tc,
        tc.tile_pool(name="tp", bufs=1) as tp,
        tc.tile_pool(name="sp", bufs=1) as sp,
        tc.tile_pool(name="cp", bufs=1) as cp,
        tc.tile_pool(name="bp", bufs=1) as bp,
        tc.tile_pool(name="c0p", bufs=1) as c0p,
        tc.tile_pool(name="gp", bufs=3) as gp,
        tc.tile_pool(name="ep", bufs=3) as ep,
        tc.tile_pool(name="pp", bufs=8, space="PSUM") as pp,
    ):
        def epilogue(cname, ct_g, c0_t, bmt, e):
            act = gp.tile([P, NW], F32, tag="act", name=f"act_{cname}_{bmt}_{e}")
            nc.scalar.activation(act[:, 0:3 * P], ct_g[:, 0:3 * P], AF.Sigmoid)
            nc.scalar.activation(act[:, 3 * P:4 * P], ct_g[:, 3 * P:4 * P], AF.Tanh)
            ct = ep.tile([P, P], F32, tag="ct", name=f"ct_{bmt}_{e}")
            nc.vector.tensor_mul(ct[:], act[:, 0:P], act[:, 3 * P:4 * P])
            fc = ep.tile([P, P], F32, tag="fc", name=f"fc_{bmt}_{e}")
            nc.vector.tensor_mul(fc[:], act[:, P:2 * P], c0_t[:])
            nc.vector.tensor_add(ct[:], ct[:], fc[:])
            nc.scalar.dma_start(out=c_out[bmt, e], in_=ct[:])
            tct = ep.tile([P, P], F32, tag="tct", name=f"tct_{bmt}_{e}")
            nc.scalar.activation(tct[:], ct[:], AF.Tanh)
            ht = ep.tile([P, P], F32, tag="ht", name=f"ht_{bmt}_{e}")
            nc.vector.tensor_mul(ht[:], act[:, 2 * P:3 * P], tct[:])
            nc.scalar.dma_start(out=h_out[bmt, e], in_=ht[:])

        s_cache = {}  # j -> (pt, tile): S tiles still resident from prior nb
        for nb in range(NB):
            first_nb = nb == 0
            tt = []
            for j in range(J):
                t = tp.tile([P, KT2, NW], F16, tag=f"tt{j}", name=f"tt_{nb}_{j}")
                q = 0
                for sz in _ksplits(KT2, first=(first_nb and j == 0), fine=first_nb):
                    nc.sync.dma_start(out=t[:, q:q + sz], in_=tTh[j, nb, :, q:q + sz])
                    q += sz
                tt.append(t)
            # zigzag pt order: the boundary pt's S tiles stay resident across nb
            pts = list(range(PT)) if nb % 2 == 0 else list(range(PT - 1, -1, -1))
            for pt in pts:
                first_unit = first_nb and pt == 0
                st = {}
                biases = {}
                c0t = {}

                def load_s(j):
                    if s_cache.get(j, (None, None))[0] == pt:
                        st[j] = s_cache[j][1]
                        return
                    s = sp.tile([P, KT2, 2 * P], F16, tag=f"st{j}", name=f"st_{nb}_{pt}_{j}")
                    q = 0
                    for sz in _ksplits(KT2, first=(first_unit and j == 0), fine=first_unit):
                        nc.scalar.dma_start(out=s[:, q:q + sz], in_=sTh[j, pt, :, q:q + sz])
                        q += sz
                    st[j] = s
                    s_cache[j] = (pt, s)

                def load_bias():
                    for half, tag in ((0, "blo"), (1, "bhi")):
                        e = half * 4 + nb
                        bt = bp.tile([P, NW], F32, tag=tag, name=f"bias_{nb}_{half}")
                        nc.scalar.dma_start(out=bt[:], in_=bb[:, e * NW:(e + 1) * NW])
                        biases[half] = bt

                def load_c0():
                    for mt2 in (0, 1):
                        mt = pt * 2 + mt2
                        for cname, (rh, eh) in CPOS.items():
                            bmt = rh * (2 * PT) + mt
                            e = eh * 4 + nb
                            t = c0p.tile([P, P], F32, tag=f"c0_{cname}_{mt2}",
                                         name=f"c0_{nb}_{pt}_{cname}_{mt2}")
                            nc.sync.dma_start(out=t[:], in_=c0b[bmt, e])
                            c0t[(cname, mt2)] = t

                # DMA issue order: critical S first, then bias (needed at the
                # first accum), then c0 (needed at the first epilogue).
                load_s(0)
                if pt == pts[0]:
                    load_bias()
                else:
                    biases = last_biases  # noqa: F821
                load_s(1)
                load_s(2)
                load_c0()
                for j in range(3, J):
                    load_s(j)
                last_biases = biases

                C = {}
                for j in range(J):
                    for mt2 in (0, 1):
                        mt = pt * 2 + mt2
                        ps = pp.tile([P, NW], F32, tag="ps", name=f"ps_{nb}_{pt}_{j}_{mt2}")
                        for k in range(KT2):
                            nc.tensor.matmul(
                                ps[:],
                                st[j][:, k, mt2 * P:(mt2 + 1) * P],
                                tt[j][:, k, :],
                                start=(k == 0),
                                stop=(k == KT2 - 1),
                            )
                        for cname, sign, first in ACC[j]:
                            key = (cname, mt2)
                            rh, eh = CPOS[cname]
                            if first:
                                ctile = cp.tile([P, NW], F32, tag=f"C{cname}_{mt2}",
                                                name=f"C_{nb}_{pt}_{cname}_{mt2}")
                                nc.vector.tensor_add(ctile[:], ps[:], biases[eh][:])
                                C[key] = ctile
                            elif sign > 0:
                                nc.vector.tensor_add(C[key][:], C[key][:], ps[:])
                            else:
                                nc.vector.tensor_sub(C[key][:], C[key][:], ps[:])
                        for cname, sign, first in ACC[j]:
                            if LAST_TOUCH[cname] == j:
                                key = (cname, mt2)
                                rh, eh = CPOS[cname]
                                bmt = rh * (2 * PT) + mt
                                e = eh * 4 + nb
                                epilogue(cname, C[key], c0t[key], bmt, e)
    nc.compile()
    return nc


def pack_inputs(y, ctx, c0, h0, W, U, C, b):
    """Host-side packing: Strassen S/T combos in fp32, rounded once to fp16."""
    x = np.concatenate([y, h0, ctx], axis=1)  # [B, KD]
    Wcat = np.concatenate([W, U, C], axis=1)  # [4H, KD]
    # packed gate column order: n = e*512 + gate*128 + u
    Bp = np.ascontiguousarray(
        Wcat.reshape(4, 8, P, KD).transpose(3, 1, 0, 2).reshape(KD, 4 * H)
    )
    B11, B12 = Bp[:KH, :2048], Bp[:KH, 2048:]
    B21, B22 = Bp[KH:, :2048], Bp[KH:, 2048:]
    T_list = [B11 + B22, B11, B12 - B22, B21 - B11, B22, B11 + B12, B21 + B22]
    tTh = np.empty((J, NB, P, KT2, 4 * P), dtype=np.float16)
    for j, T in enumerate(T_list):
        # T: [KH, 2048] -> [nb, p, kt, w]
        tTh[j] = T.reshape(KT2, P, NB, 4 * P).transpose(2, 1, 0, 3).astype(np.float16)

    sThs = []
    for ci in range(NCORES):
        xc = x[ci * BC:(ci + 1) * BC]
        A11, A12 = xc[:MH, :KH], xc[:MH, KH:]
        A21, A22 = xc[MH:, :KH], xc[MH:, KH:]
        S_list = [A11 + A22, A21 + A22, A11, A22, A11 + A12, A21 - A11, A12 - A22]
        sTh = np.empty((J, PT, P, KT2, 2 * P), dtype=np.float16)
        for j, S in enumerate(S_list):
            # S: [MH, KH] -> [pt, p, kt, u]
            sTh[j] = S.reshape(PT, 2 * P, KT2, P).transpose(0, 3, 2, 1).astype(np.float16)
        sThs.append(sTh)

    br = b.reshape(4, 8, P).transpose(1, 0, 2).reshape(4 * H)
    bb = np.ascontiguousarray(np.broadcast_to(br, (P, 4 * H)))

    c0bs = []
    for ci in range(NCORES):
        c0c = c0[ci * BC:(ci + 1) * BC]
        c0bs.append(np.ascontiguousarray(
            c0c.reshape(2 * PT * 2, P, 8, P).transpose(0, 2, 1, 3)
        ))
    return sThs, tTh, bb, c0bs


def kernel(y, ctx, c0, h0, W, U, C, b):
    global LAST_RESULT, _NC_CACHE
    y = np.ascontiguousarray(np.asarray(y, dtype=np.float32))
    ctx = np.ascontiguousarray(np.asarray(ctx, dtype=np.float32))
    c0 = np.ascontiguousarray(np.asarray(c0, dtype=np.float32))
    h0 = np.ascontiguousarray(np.asarray(h0, dtype=np.float32))
    W = np.ascontiguousarray(np.asarray(W, dtype=np.float32))
    U = np.ascontiguousarray(np.asarray(U, dtype=np.float32))
    C = np.ascontiguousarray(np.asarray(C, dtype=np.float32))
    b = np.ascontiguousarray(np.asarray(b, dtype=np.float32))

    sThs, tTh, bb, c0bs = pack_inputs(y, ctx, c0, h0, W, U, C, b)

    if _NC_CACHE is None:
        _NC_CACHE = build_nc()
    nc = _NC_CACHE
    in_maps = []
    for ci in range(NCORES):
        in_maps.append(
            {
                "sTh": sThs[ci],
                "tTh": tTh,
                "c0b": c0bs[ci],
                "bb": bb,
            }
        )
    res = bass_utils.run_bass_kernel_spmd(nc, in_maps, core_ids=list(range(NCORES)))
    LAST_RESULT = res

    def unblock(a):  # [16, 8, 128, 128] -> [2048, 1024]
        return a.transpose(0, 2, 1, 3).reshape(BC, H)

    c_full = np.concatenate([unblock(r["c_out"]) for r in res.results], axis=0)
    h_full = np.concatenate([unblock(r["h_out"]) for r in res.results], axis=0)
    return (c_full, h_full)


# revision 17
# speedup vs baseline: 1.1840x; 1.1804x over previous
"""Trainium2 Bass kernel for nn_DecoderLSTMCell.

Computes, for B=16384 rows:
    gates = y @ W.T + h0 @ U.T + ctx @ C.T + b            # [B, 4H]
    i, f, o, g = split(gates, 4); i,f,o = sigmoid; g = tanh
    c = i * g + f * c0 ; h = o * tanh(c)
Returns (c, h), both [B, H] float32.

Strategy: data-parallel over the batch dim across 8 NeuronCores (2048
rows/core), weights replicated.  Per core the gate GEMM is
[M=2048, K=4096] @ [K=4096, N=4096] — computed via ONE level of
Strassen with fp16 operands:

  * fp16 runs the tensor engine at the same 1 cycle/row as bf16 but with
    8x less rounding noise, which buys the error headroom Strassen needs
    (measured ~2.4e-3 max rel err vs the 2e-2 gate; plain bf16 is 8e-3).
  * Strassen does 7 half-size products instead of 8: 3584 matmul
    instructions instead of 4096, i.e. 7/8 of the tensor-engine time,
    which is the bottleneck (96% busy in the direct kernel).

The 7 S (x-side) and 7 T (weight-side) block combinations are formed on
the host in fp32 and shipped as fp16 — free accuracy and zero device
cost.  On-device, each (nb, mt-pair) unit accumulates the 7 products
into four SBUF gate tiles (C11/C12/C21/C22) via DVE adds with the bias
folded into the first touch, then runs the LSTM epilogue per gate tile
as soon as its last product lands.
"""

import numpy as np

import concourse.tile as tile
import concourse.mybir as mybir
from concourse import bacc, bass_utils

P = 128
F32 = mybir.dt.float32
F16 = mybir.dt.float16
AF = mybir.ActivationFunctionType

# Problem shapes (hardcoded; see module docstring)
B, IN, H, CTX = 16384, 1024, 1024, 2048
KD = IN + H + CTX  # 4096 contraction dim
NCORES = 8
BC = B // NCORES  # 2048 batch rows per core
MH = BC // 2      # 1024 = Strassen row-block
KH = KD // 2      # 2048 = Strassen contraction block
KT2 = KH // P     # 16 k-tiles per product
NB = 4            # 512-wide gate blocks per N-half
PT = 4            # mt-pairs per row-half
J = 7             # Strassen products

# accumulation plan: per product j, list of (Cname, sign, is_first_touch);
# epilogue fires at each C tile's last touch.
#   C11 = P0+P3-P4+P6 ; C12 = P2+P4 ; C21 = P1+P3 ; C22 = P0-P1+P2+P5
ACC = [
    [("c11", 1, True), ("c22", 1, True)],
    [("c21", 1, True), ("c22", -1, False)],
    [("c12", 1, True), ("c22", 1, False)],
    [("c11", 1, False), ("c21", 1, False)],
    [("c12", 1, False), ("c11", -1, False)],
    [("c22", 1, False)],
    [("c11", 1, False)],
]
LAST_TOUCH = {"c21": 3, "c12": 4, "c22": 5, "c11": 6}
# (row_half, e_half) per C name: rows = row_half*MH + mt*P, e = e_half*4 + nb
CPOS = {"c11": (0, 0), "c21": (1, 0), "c12": (0, 1), "c22": (1, 1)}

LAST_RESULT = None  # BassKernelResults of the most recent run (for test.py)
_NC_CACHE = None  # compiled Bass module, reused across kernel() calls


def _ksplits(kt, first, fine=False):
    """k-tile DMA split sizes; fine-grained ramp on the critical first load.
    `fine` keeps chunks small so warmup stalls stay under the ~3.4us HAM MID
    window (a longer PE idle re-throttles the clock gate to 1.2 GHz)."""
    if first:
        return [1, 1, 2, 4, kt - 8]
    if fine:
        return [4] * (kt // 4)
    return [kt // 4] * 4


def build_nc():
    nc = bacc.Bacc("TRN2", target_bir_lowering=False)
    sTh = nc.dram_tensor("sTh", (J, PT, P, KT2, 2 * P), F16, kind="ExternalInput")
    tTh = nc.dram_tensor("tTh", (J, NB, P, KT2, 4 * P), F16, kind="ExternalInput")
    # c0 / outputs in blocked [mt, e, 128, 128] layout: contiguous 64KB DMA
    # tiles (vs 512B strided bursts of the flat [BC, H] layout)
    c0b = nc.dram_tensor("c0b", (2 * PT * 2, 8, P, P), F32, kind="ExternalInput")
    bb = nc.dram_tensor("bb", (P, 4 * H), F32, kind="ExternalInput")
    c_out = nc.dram_tensor("c_out", (2 * PT * 2, 8, P, P), F32, kind="ExternalOutput")
    h_out = nc.dram_tensor("h_out", (2 * PT * 2, 8, P, P), F32, kind="ExternalOutput")
    NW = 4 * P  # 512: one [i|f|o|g] gate block

    with (
        tile.TileContext(nc) as tc,
        tc.tile_pool(name="tp", bufs=1) as tp,
        tc.tile_pool(name="sp", bufs=1) as sp,
        tc.tile_pool(name="cp", bufs=1) as cp,
        tc.tile_pool(name="bp", bufs=1) as bp,
        tc.tile_pool(name="c0p", bufs=1) as c0p,
        tc.tile_pool(name="gp", bufs=3) as gp,
        tc.tile_pool(name="ep", bufs=3) as ep,
        tc.tile_pool(name="pp", bufs=8, space="PSUM") as pp,
    ):
        def epilogue(cname, ct_g, c0_t, bmt, e):
            act = gp.tile([P, NW], F32, tag="act", name=f"act_{cname}_{bmt}_{e}")
            nc.scalar.activation(act[:, 0:3 * P], ct_g[:, 0:3 * P], AF.Sigmoid)
            nc.scalar.activation(act[:, 3 * P:4 * P], ct_g[:, 3 * P:4 * P], AF.Tanh)
            ct = ep.tile([P, P], F32, tag="ct", name=f"ct_{bmt}_{e}")
            nc.vector.tensor_mul(ct[:], act[:, 0:P], act[:, 3 * P:4 * P])
            fc = ep.tile([P, P], F32, tag="fc", name=f"fc_{bmt}_{e}")
            nc.vector.tensor_mul(fc[:], act[:, P:2 * P], c0_t[:])
            nc.vector.tensor_add(ct[:], ct[:], fc[:])
            nc.scalar.dma_start(out=c_out[bmt, e], in_=ct[:])
            tct = ep.tile([P, P], F32, tag="tct", name=f"tct_{bmt}_{e}")
            nc.scalar.activation(tct[:], ct[:], AF.Tanh)
            ht = ep.tile([P, P], F32, tag="ht", name=f"ht_{bmt}_{e}")
            nc.vector.tensor_mul(ht[:], act[:, 2 * P:3 * P], tct[:])
            nc.scalar.dma_start(out=h_out[bmt, e], in_=ht[:])

        s_cache = {}  # j -> (pt, tile): S tiles still resident from prior nb
        for nb in range(NB):
            first_nb = nb == 0
            tt = []
            for j in range(J):
                t = tp.tile([P, KT2, NW], F16, tag=f"tt{j}", name=f"tt_{nb}_{j}")
                q = 0
                for sz in _ksplits(KT2, first=(first_nb and j == 0), fine=first_nb):
                    nc.sync.dma_start(out=t[:, q:q + sz], in_=tTh[j, nb, :, q:q + sz])
                    q += sz
                tt.append(t)
            # zigzag pt order: the boundary pt's S tiles stay resident across nb
            pts = list(range(PT)) if nb % 2 == 0 else list(range(PT - 1, -1, -1))
            for pt in pts:
                first_unit = first_nb and pt == 0
                st = {}
                biases = {}
                c0t = {}

                def load_s(j):
                    if s_cache.get(j, (None, None))[0] == pt:
                        st[j] = s_cache[j][1]
                        return
                    s = sp.tile([P, KT2, 2 * P], F16, tag=f"st{j}", name=f"st_{nb}_{pt}_{j}")
                    q = 0
                    for sz in _ksplits(KT2, first=(first_unit and j == 0), fine=first_unit):
                        nc.scalar.dma_start(out=s[:, q:q + sz], in_=sTh[j, pt, :, q:q + sz])
                        q += sz
                    st[j] = s
                    s_cache[j] = (pt, s)

                def load_bias():
                    for half, tag in ((0, "blo"), (1, "bhi")):
                        e = half * 4 + nb
                        bt = bp.tile([P, NW], F32, tag=tag, name=f"bias_{nb}_{half}")
                        nc.scalar.dma_start(out=bt[:], in_=bb[:, e * NW:(e + 1) * NW])
                        biases[half] = bt

                def load_c0():
                    for mt2 in (0, 1):
                        mt = pt * 2 + mt2
                        for cname, (rh, eh) in CPOS.items():
                            bmt = rh * (2 * PT) + mt
                            e = eh * 4 + nb
                            t = c0p.tile([P, P], F32, tag=f"c0_{cname}_{mt2}",
                                         name=f"c0_{nb}_{pt}_{cname}_{mt2}")
                            nc.sync.dma_start(out=t[:], in_=c0b[bmt, e])
                            c0t[(cname, mt2)] = t

                # DMA issue order: critical S first, then bias (needed at the
                # first accum), then c0 (needed at the first epilogue).
                load_s(0)
                if pt == pts[0]:
                    load_bias()
                else:
                    biases = last_biases  # noqa: F821
                load_s(1)
                load_s(2)
                load_c0()
                for j in range(3, J):
                    load_s(j)
                last_biases = biases

                C = {}
                for j in range(J):
                    for mt2 in (0, 1):
                        mt = pt * 2 + mt2
                        ps = pp.tile([P, NW], F32, tag="ps", name=f"ps_{nb}_{pt}_{j}_{mt2}")
                        for k in range(KT2):
                            nc.tensor.matmul(
                                ps[:],
                                st[j][:, k, mt2 * P:(mt2 + 1) * P],
                                tt[j][:, k, :],
                                start=(k == 0),
                                stop=(k == KT2 - 1),
                            )
                        for cname, sign, first in ACC[j]:
                            key = (cname, mt2)
                            rh, eh = CPOS[cname]
                            if first:
                                ctile = cp.tile([P, NW], F32, tag=f"C{cname}_{mt2}",
                                                name=f"C_{nb}_{pt}_{cname}_{mt2}")
                                nc.vector.tensor_add(ctile[:], ps[:], biases[eh][:])
                                C[key] = ctile
                            elif sign > 0:
                                nc.vector.tensor_add(C[key][:], C[key][:], ps[:])
                            else:
                                nc.vector.tensor_sub(C[key][:], C[key][:], ps[:])
                        for cname, sign, first in ACC[j]:
                            if LAST_TOUCH[cname] == j:
                                key = (cname, mt2)
                                rh, eh = CPOS[cname]
                                bmt = rh * (2 * PT) + mt
                                e = eh * 4 + nb
                                epilogue(cname, C[key], c0t[key], bmt, e)
    nc.compile()
    return nc


def pack_inputs(y, ctx, c0, h0, W, U, C, b):
    """Host-side packing: Strassen S/T combos in fp32, rounded once to fp16."""
    x = np.concatenate([y, h0, ctx], axis=1)  # [B, KD]
    Wcat = np.concatenate([W, U, C], axis=1)  # [4H, KD]
    # packed gate column order: n = e*512 + gate*128 + u
    Bp = np.ascontiguousarray(
        Wcat.reshape(4, 8, P, KD).transpose(3, 1, 0, 2).reshape(KD, 4 * H)
    )
    B11, B12 = Bp[:KH, :2048], Bp[:KH, 2048:]
    B21, B22 = Bp[KH:, :2048], Bp[KH:, 2048:]
    T_list = [B11 + B22, B11, B12 - B22, B21 - B11, B22, B11 + B12, B21 + B22]
    tTh = np.empty((J, NB, P, KT2, 4 * P), dtype=np.float16)
    for j, T in enumerate(T_list):
        # T: [KH, 2048] -> [nb, p, kt, w]
        tTh[j] = T.reshape(KT2, P, NB, 4 * P).transpose(2, 1, 0, 3).astype(np.float16)

    sThs = []
    for ci in range(NCORES):
        xc = x[ci * BC:(ci + 1) * BC]
        A11, A12 = xc[:MH, :KH], xc[:MH, KH:]
        A21, A22 = xc[MH:, :KH], xc[MH:, KH:]
        S_list = [A11 + A22, A21 + A22, A11, A22, A11 + A12, A21 - A11, A12 - A22]
        sTh = np.empty((J, PT, P, KT2, 2 * P), dtype=np.float16)
        for j, S in enumerate(S_list):
            # S: [MH, KH] -> [pt, p, kt, u]
            sTh[j] = S.reshape(PT, 2 * P, KT2, P).transpose(0, 3, 2, 1).astype(np.float16)
        sThs.append(sTh)

    br = b.reshape(4, 8, P).transpose(1, 0, 2).reshape(4 * H)
    bb = np.ascontiguousarray(np.broadcast_to(br, (P, 4 * H)))

    c0bs = []
    for ci in range(NCORES):
        c0c = c0[ci * BC:(ci + 1) * BC]
        c0bs.append(np.ascontiguousarray(
            c0c.reshape(2 * PT * 2, P, 8, P).transpose(0, 2, 1, 3)
        ))
    return sThs, tTh, bb, c0bs


def kernel(y, ctx, c0, h0, W, U, C, b):
    global LAST_RESULT, _NC_CACHE
    y = np.ascontiguousarray(np.asarray(y, dtype=np.float32))
    ctx = np.ascontiguousarray(np.asarray(ctx, dtype=np.float32))
    c0 = np.ascontiguousarray(np.asarray(c0, dtype=np.float32))
    h0 = np.ascontiguousarray(np.asarray(h0, dtype=np.float32))
    W = np.ascontiguousarray(np.asarray(W, dtype=np.float32))
    U = np.ascontiguousarray(np.asarray(U, dtype=np.float32))
    C = np.ascontiguousarray(np.asarray(C, dtype=np.float32))
    b = np.ascontiguousarray(np.asarray(b, dtype=np.float32))

    sThs, tTh, bb, c0bs = pack_inputs(y, ctx, c0, h0, W, U, C, b)

    if _NC_CACHE is None:
        _NC_CACHE = build_nc()
    nc = _NC_CACHE
    in_maps = []
    for ci in range(NCORES):
        in_maps.append(
            {
                "sTh": sThs[ci],
                "tTh": tTh,
                "c0b": c0bs[ci],
                "bb": bb,
            }
        )
    res = bass_utils.run_bass_kernel_spmd(nc, in_maps, core_ids=list(range(NCORES)))
    LAST_RESULT = res

    def unblock(a):  # [16, 8, 128, 128] -> [2048, 1024]
        return a.transpose(0, 2, 1, 3).reshape(BC, H)

    c_full = np.concatenate([unblock(r["c_out"]) for r in res.results], axis=0)
    h_full = np.concatenate([unblock(r["h_out"]) for r in res.results], axis=0)
    return (c_full, h_full)
